# revision 35
# baseline (speedup 1.0000x reference)
"""Trainium2 Bass kernel for nn_Model_14499809591701 (gnn_message_passing).

Math (matching reference.py):
    x_enc = leaky(leaky(x@W1_0.T+b1_0)@W1_1.T+b1_1)          # [N, 64]
    s     = x_enc @ a                                         # [N]
    Sraw  = exp(leaky(s_i - s_j)) * A                         # [N, N]
    rho_i = sum_j Sraw_ij ;  Sn = Sraw / rho
    sq_i  = |x_enc_i|^2 / 64
    L1    = (1/N^2) sum_i [ sq_i + (Sn@sq)_i - (2/64) <x_enc_i, (Sn@x_enc)_i> ]
    L2    = (1/N^2) sum_i sum_j Sraw_ij^2 / rho_i^2
    y     = Sn @ x_enc ;  out = leaky(y@Wg.T+bg) @ W2.T + b2  # [N, 32]

(L1 uses sum_j Sn_ij = 1 so no [N,N] pair_mse tensor is needed; the head uses
leaky's positive homogeneity so the 1/rho normalization is applied after the
final transpose as a per-partition ACT scale, with rank-1 rho x bias terms
folded into the head matmuls as K=1 accumulations.)

Sharding: rows i are split across 8 cores (R = N/8 rows each), no collectives.
Each core duplicates the cheap encoder on full x; the main loop walks j-blocks
of 128: A-blocks are PE-transposed to PSUM, multiplied by
E^T = exp(leaky(s_i - s_j)) built directly in [j-part, i-free] layout
(ACT bias = -s_j per partition), and a PE matmul with lhsT = X_aug[j-block]
(97 cols: x_enc | ones@64 -> rho | zero gap | sq@96 -> Sraw@sq) accumulates
y^T / rho / Sraw@sq in PSUM; a ones-matmul accumulates sum_j Sraw^2.

PSUM discipline (walrus limit: a Matmult carries at most ~2 sync waits, a
transpose-Matmult exactly 1): PSUM is managed as four permanently-allocated
[128, 1024] tensors (big_a..big_d) sliced by each phase — no pool recycling,
so no zone-release sems land on PE instructions. Before each transpose group a
"mini" 1-wait matmul on the identity absorbs the single cross-engine WAR dep.
"""

import os
import sys

for _p in ("/opt/trn_rl_repo", os.path.expanduser("~/.axon_site/_ro/trn_rl_repo")):
    if os.path.isdir(_p) and _p not in sys.path:
        sys.path.insert(0, _p)

from contextlib import ExitStack

import numpy as np

import concourse.bacc as bacc
import concourse.bass as bass
import concourse.mybir as mybir
import concourse.tile as tile

F32 = mybir.dt.float32
AF = mybir.ActivationFunctionType
ALU = mybir.AluOpType

P = 128        # partitions
D_IN = 256     # encoder input dim
H1 = 128       # encoder hidden
H2 = 64        # encoded dim
C_AUG = H2 + 33  # x_enc | ones(->rho @ p64) | zero gap | sq(->Sraw@sq @ p96)
D_OUT = 32
ALPHA = 0.1    # leaky slope


def _mini(nc, corner, ident):
    """1-wait PE matmul writing a PSUM corner: absorbs the cross-engine WAR
    dep so following transpose matmuls carry only their own input wait."""
    nc.tensor.matmul(corner, ident[:, 0:1], ident[:, 0:1], start=True,
                     stop=True)


def _encoder(nc, pools, x_ext, T, consts, xeT, pb,
             s_col=None, sq_col=None, xaug=None,
             s_row_psum=None, sq_row_psum=None):
    """Encode x_ext [T, 256] -> xeT [64, T] (SBUF, transposed).

    pb: psum slices {pxT0, pxT0_c, pxT1, pxT1_c, ph1, ph2, pxa, pxa_c}.
    """
    ident, W10T, b10, W11T, b11, a_c, inv64 = (
        consts["ident"], consts["W10T"], consts["b10"], consts["W11T"],
        consts["b11"], consts["a_c"], consts["inv64"])
    io, sb = pools["io"], pools["sb"]

    chunk = min(512, T)
    ntile = chunk // P
    for st in range(T // chunk):
        c0 = st * chunk
        xts = []
        for t in range(ntile):
            xt = io.tile([P, D_IN], F32, tag="enc_xt")
            nc.sync.dma_start(xt[:], x_ext[c0 + t * P: c0 + (t + 1) * P, :])
            xts.append(xt)
        # transpose x chunk -> xT [256, chunk] as two [128, chunk] k-tiles
        mm_xts = []
        for k in range(2):
            pxT = pb[f"pxT{k}"]
            _mini(nc, pb[f"pxT{k}_c"], ident)
            for t in range(ntile):
                nc.tensor.transpose(pxT[:, t * P:(t + 1) * P],
                                    xts[t][:, k * P:(k + 1) * P], ident[:])
            sxT = sb.tile([P, chunk], F32, tag=f"enc_sxT{k}")
            nc.scalar.copy(sxT[:], pxT[:, 0:chunk])
            mm_xts.append(sxT)
        # h1T = leaky(W1_0 @ xT + b1_0)
        ph1 = pb["ph1"]
        for k in range(2):
            nc.tensor.matmul(ph1[:, 0:chunk], W10T[:, k * P:(k + 1) * P],
                             mm_xts[k][:], start=(k == 0), stop=(k == 1))
        uh1 = sb.tile([P, chunk], F32, tag="enc_uh1")
        nc.scalar.activation(uh1[:], ph1[:, 0:chunk], AF.Identity, bias=b10[:])
        rh1 = sb.tile([P, chunk], F32, tag="enc_rh1")
        nc.scalar.activation(rh1[:], ph1[:, 0:chunk], AF.Relu,
                             bias=consts["nb10"][:], scale=-1.0)
        h1 = sb.tile([P, chunk], F32, tag="enc_h1")
        nc.vector.scalar_tensor_tensor(h1[:], rh1[:], 0.9, uh1[:],
                                       ALU.mult, ALU.add)
        # h2T = leaky(W1_1 @ h1T + b1_1) -> xe chunk
        ph2 = pb["ph2"]
        nc.tensor.matmul(ph2[:, 0:chunk], W11T[:], h1[:], start=True,
                         stop=True)
        if xeT is not None:
            xe_chunk = xeT[:, c0:c0 + chunk]
        else:
            xe_t = sb.tile([H2, chunk], F32, tag="enc_xeC")
            xe_chunk = xe_t[:]
        uxe = sb.tile([H2, chunk], F32, tag="enc_uxe")
        nc.scalar.activation(uxe[:], ph2[:, 0:chunk], AF.Identity, bias=b11[:])
        rxe = sb.tile([H2, chunk], F32, tag="enc_rxe")
        nc.scalar.activation(rxe[:], ph2[:, 0:chunk], AF.Relu,
                             bias=consts["nb11"][:], scale=-1.0)
        nc.vector.scalar_tensor_tensor(xe_chunk, rxe[:], 0.9, uxe[:],
                                       ALU.mult, ALU.add)
        xe2 = None
        if sq_col is not None or sq_row_psum is not None:
            xe2 = sb.tile([H2, chunk], F32, tag="enc_xe2")
            nc.scalar.activation(xe2[:], xe_chunk, AF.Square)
        if s_col is not None:
            for q in range(ntile):
                jb = st * ntile + q
                nc.tensor.matmul(s_col[:, jb:jb + 1],
                                 xe_chunk[:, q * P:(q + 1) * P], a_c[:],
                                 start=True, stop=True)
        if sq_col is not None:
            for q in range(ntile):
                jb = st * ntile + q
                nc.tensor.matmul(sq_col[:, jb:jb + 1],
                                 xe2[:, q * P:(q + 1) * P], inv64[:],
                                 start=True, stop=True)
        if xaug is not None:
            pxa = pb["pxa"]
            _mini(nc, pb["pxa_c"], ident)
            for q in range(ntile):
                nc.tensor.transpose(pxa[:, q * H2:(q + 1) * H2],
                                    xe_chunk[:, q * P:(q + 1) * P],
                                    ident[0:H2, 0:H2])
            jb0 = st * ntile
            dst = xaug[:].rearrange("p (b c) -> p b c", c=C_AUG)
            src = pxa[:, 0:ntile * H2].rearrange("p (b c) -> p b c", c=H2)
            nc.scalar.copy(dst[:, jb0:jb0 + ntile, 0:H2], src[:])
        if s_row_psum is not None:
            for f in range(0, chunk, 512):
                w = min(512, chunk - f)
                nc.tensor.matmul(s_row_psum[:, c0 + f:c0 + f + w], a_c[:],
                                 xe_chunk[:, f:f + w], start=True, stop=True)
        if sq_row_psum is not None:
            for f in range(0, chunk, 512):
                w = min(512, chunk - f)
                nc.tensor.matmul(sq_row_psum[:, c0 + f:c0 + f + w], inv64[:],
                                 xe2[:, f:f + w], start=True, stop=True)


def build_program(N, R):
    """Build the single-core SPMD Bass program. N nodes total, R rows/core."""
    nc = bacc.Bacc("TRN2")
    NJ = N // P           # number of j-blocks
    NT = R // P           # number of i-tiles in this core's row range
    NF = min(512, R)      # matmul moving free-dim chunk

    x_full = nc.declare_dram_parameter("x_full", [N, D_IN], F32, isOutput=False)
    x_mine = nc.declare_dram_parameter("x_mine", [R, D_IN], F32, isOutput=False)
    A_sh = nc.declare_dram_parameter("A_sh", [R, N], F32, isOutput=False)
    W1_0 = nc.declare_dram_parameter("W1_0", [H1, D_IN], F32, isOutput=False)
    b1_0 = nc.declare_dram_parameter("b1_0", [H1, 1], F32, isOutput=False)
    W1_1 = nc.declare_dram_parameter("W1_1", [H2, H1], F32, isOutput=False)
    b1_1 = nc.declare_dram_parameter("b1_1", [H2, 1], F32, isOutput=False)
    a_p = nc.declare_dram_parameter("a_vec", [H2, 1], F32, isOutput=False)
    Wg = nc.declare_dram_parameter("Wg", [H2, H2], F32, isOutput=False)
    bg = nc.declare_dram_parameter("bg", [1, H2], F32, isOutput=False)
    W2 = nc.declare_dram_parameter("W2", [D_OUT, H2], F32, isOutput=False)
    b2 = nc.declare_dram_parameter("b2", [1, D_OUT], F32, isOutput=False)
    ident_p = nc.declare_dram_parameter("ident", [P, P], F32, isOutput=False)
    out_mine = nc.declare_dram_parameter("out_mine", [R, D_OUT], F32,
                                         isOutput=True)
    partials = nc.declare_dram_parameter("partials", [1, 2], F32, isOutput=True)

    with tile.TileContext(nc) as tc, ExitStack() as top:
        const = top.enter_context(tc.tile_pool(name="const", bufs=1))
        persist = top.enter_context(tc.tile_pool(name="persist", bufs=1))
        esb = top.enter_context(tc.tile_pool(name="e_sb", bufs=1))
        psg = top.enter_context(tc.tile_pool(name="psg", bufs=1, space="PSUM"))

        # ---- the four permanent PSUM tensors (2 banks each = all 8) ----
        big_a = psg.tile([P, 1024], F32, tag="big_a")
        big_b = psg.tile([P, 1024], F32, tag="big_b")
        big_c = psg.tile([P, 1024], F32, tag="big_c")
        big_d = psg.tile([P, 1024], F32, tag="big_d")

        # ---- constants into SBUF ----
        ident = const.tile([P, P], F32, tag="ident")
        nc.sync.dma_start(ident[:], ident_p[:])
        w10 = const.tile([H1, D_IN], F32, tag="w10")
        nc.sync.dma_start(w10[:], W1_0[:])
        w11 = const.tile([H2, H1], F32, tag="w11")
        nc.sync.dma_start(w11[:], W1_1[:])
        wg = const.tile([H2, H2], F32, tag="wg")
        nc.sync.dma_start(wg[:], Wg[:])
        w2 = const.tile([D_OUT, H2], F32, tag="w2")
        nc.sync.dma_start(w2[:], W2[:])
        b10 = const.tile([H1, 1], F32, tag="b10")
        nc.sync.dma_start(b10[:], b1_0[:])
        b11 = const.tile([H2, 1], F32, tag="b11")
        nc.sync.dma_start(b11[:], b1_1[:])
        bgr = const.tile([1, H2], F32, tag="bgr")
        nc.sync.dma_start(bgr[:], bg[:])
        b2r = const.tile([1, D_OUT], F32, tag="b2r")
        nc.sync.dma_start(b2r[:], b2[:])
        a_c = const.tile([H2, 1], F32, tag="a_c")
        nc.sync.dma_start(a_c[:], a_p[:])
        inv64 = const.tile([H2, 1], F32, tag="inv64")
        nc.gpsimd.memset(inv64[:], 1.0 / H2)
        ones128 = const.tile([P, 1], F32, tag="ones128")
        nc.gpsimd.memset(ones128[:], 1.0)
        ones64c = const.tile([H2, 1], F32, tag="ones64c")
        nc.gpsimd.memset(ones64c[:], 1.0)
        ones1r128 = const.tile([1, P], F32, tag="ones1r128")
        nc.gpsimd.memset(ones1r128[:], 1.0)

        # ---- PE warmup: observe const-producer sems one at a time ----
        nc.tensor.matmul(big_d[0:1, 0:P], ident[:, 0:1], ident[:],
                         start=True, stop=True)
        nc.tensor.matmul(big_d[0:1, 0:1], a_c[:, 0:1], a_c[:],
                         start=True, stop=True)
        nc.tensor.matmul(big_d[0:1, 0:1], inv64[:, 0:1], inv64[:],
                         start=True, stop=True)
        nc.tensor.matmul(big_d[0:1, 0:1], ones128[:, 0:1], ones128[:],
                         start=True, stop=True)

        # ---- transposed weights via PE (staged in big_a) ----
        nc.tensor.transpose(big_a[:, 0:P], w10[:, 0:P], ident[:])
        nc.tensor.transpose(big_a[:, P:2 * P], w10[:, P:2 * P], ident[:])
        nc.tensor.transpose(big_a[:, 2 * P:2 * P + H2], w11[:],
                            ident[0:H2, 0:H2])
        nc.tensor.transpose(big_a[0:H2, 320:320 + H2], wg[:],
                            ident[0:H2, 0:H2])
        nc.tensor.transpose(big_a[0:H2, 384:384 + D_OUT], w2[:],
                            ident[0:D_OUT, 0:D_OUT])
        # WALL layout: [W10T (256) | W11T (64) | WgT (64) | W2T (32)]
        WALL = const.tile([P, 2 * P + H2 + H2 + D_OUT], F32, tag="WALL")
        nc.scalar.copy(WALL[:, 0:2 * P + H2], big_a[:, 0:2 * P + H2])
        nc.scalar.copy(WALL[0:H2, 2 * P + H2:],
                              big_a[0:H2, 320:320 + H2 + D_OUT])
        W10T = WALL[:, 0:2 * P]
        W11T = WALL[:, 2 * P:2 * P + H2]
        WgT = WALL[0:H2, 2 * P + H2:2 * P + 2 * H2]
        W2T = WALL[0:H2, 2 * P + 2 * H2:2 * P + 2 * H2 + D_OUT]

        nb10 = const.tile([H1, 1], F32, tag="nb10")
        nc.vector.tensor_scalar(nb10[:], b10[:], -1.0, None, ALU.mult)
        nb11 = const.tile([H2, 1], F32, tag="nb11")
        nc.vector.tensor_scalar(nb11[:], b11[:], -1.0, None, ALU.mult)
        consts = dict(ident=ident, W10T=W10T, b10=b10, W11T=W11T, b11=b11,
                      a_c=a_c, inv64=inv64, nb10=nb10, nb11=nb11)

        # ---- persistent SBUF tensors ----
        xmT = persist.tile([H2, R], F32, tag="xmT")
        xaug = persist.tile([P, NJ * C_AUG], F32, tag="xaug")
        s_colA = persist.tile([P, NJ], F32, tag="s_colA")
        sq_colA = persist.tile([P, NJ], F32, tag="sq_colA")
        negs_col = persist.tile([P, NJ], F32, tag="negs_col")
        negs01_col = persist.tile([P, NJ], F32, tag="negs01_col")
        s_i_rep = persist.tile([P, R], F32, tag="s_i_rep")
        sqm_row = persist.tile([1, R], F32, tag="sqm_row")
        sm_row = persist.tile([1, R], F32, tag="sm_row")

        pb = {
            "pxT0": big_b[:, 0:512], "pxT0_c": big_b[0:1, 0:1],
            "pxT1": big_b[:, 512:1024], "pxT1_c": big_b[0:1, 512:513],
            "ph1": big_c[:, 0:512],
            "ph2": big_c[0:H2, 512:1024],
            "pxa": big_a[:, 0:512], "pxa_c": big_a[0:1, 0:1],
        }

        # ---- phase 1: encoders ----
        # (pools live in top scope: recycling their SBUF into the main loop
        #  would attach the encoder's last-reader WAR sems to main-loop
        #  transposes, which can only carry one sync wait)
        with ExitStack() as ctx1:
            pools = {
                "io": top.enter_context(tc.tile_pool(name="enc_io", bufs=12)),
                "sb": top.enter_context(tc.tile_pool(name="enc_sb", bufs=2)),
            }
            ps_scol = big_d[:, 256:256 + NJ]
            ps_sqcol = big_d[:, 384:384 + NJ]
            _encoder(nc, pools, x_full, N, consts, None, pb,
                     s_col=ps_scol, sq_col=ps_sqcol, xaug=xaug)
            nc.scalar.copy(s_colA[:], ps_scol)
            nc.scalar.copy(sq_colA[:], ps_sqcol)

            ps_srow = big_d[0:1, 0:R]
            ps_sqrow = big_d[32:33, 0:R]
            _encoder(nc, pools, x_mine, R, consts, xmT, pb,
                     s_row_psum=ps_srow, sq_row_psum=ps_sqrow)
            nc.scalar.copy(sqm_row[:], ps_sqrow)
            nc.scalar.copy(sm_row[:], ps_srow)

            # small derived tensors
            nc.vector.tensor_scalar(negs_col[:], s_colA[:], -1.0, None,
                                    ALU.mult)
            nc.vector.tensor_scalar(negs01_col[:], s_colA[:], -0.1, None,
                                    ALU.mult)
            _mini(nc, big_a[0:1, 0:1], ident)
            for f in range(0, R, 512):
                w = min(512, R - f)
                nc.tensor.matmul(big_a[:, f:f + w], ones1r128[:],
                                 sm_row[:, f:f + w], start=True, stop=True)
            nc.scalar.copy(s_i_rep[:], big_a[:, 0:R])
            xaug3 = xaug[:].rearrange("p (b c) -> p b c", c=C_AUG)
            nc.vector.memset(xaug3[:, :, H2:C_AUG], 0.0)
            nc.vector.memset(xaug3[:, :, H2:H2 + 1], 1.0)
            nc.vector.tensor_copy(xaug3[:, :, C_AUG - 1:C_AUG],
                                  sq_colA[:].rearrange("p (b c) -> p b c", c=1))

        # ---- phase 2: main loop over j-blocks ----
        psum_y = big_c[0:C_AUG, 0:R]
        psum_ssq = big_d[0:1, 0:R]
        pats = [big_a, big_b]
        # one-time all-engine barrier: advances every engine's observed
        # vector clock past the whole prologue so the first loop iterations'
        # transpose matmuls (single-sync-wait LDW lowering) carry only their
        # own DMA wait.
        with tc.tile_critical():
            nc.vector.memset(big_a[0:1, 0:1], 0.0)
        JSUP = 1024 if N >= 1024 else N   # j-columns per A panel (4KB rows)
        JPB = JSUP // P                   # j-blocks per panel set
        with ExitStack() as ctx2:
            mio = ctx2.enter_context(
                tc.tile_pool(name="m_io", bufs=max(2, (5 * NT) // 4)))
            msb = ctx2.enter_context(tc.tile_pool(name="m_sb", bufs=2))
            panels = {}
            for jb in range(NJ):
                js, jbr = divmod(jb, JPB)
                if jbr == 0:
                    panels = {}
                    for t in range(NT):
                        pn = mio.tile([P, JSUP], F32, tag="apanel")
                        nc.sync.dma_start(
                            pn[:],
                            A_sh[t * P:(t + 1) * P,
                                 js * JSUP:(js + 1) * JSUP])
                        panels[t] = pn
                pAT = pats[jb % 2]
                _mini(nc, pAT[0:1, 0:1], ident)
                for t in range(NT):
                    nc.tensor.transpose(pAT[:, t * P:(t + 1) * P],
                                        panels[t][:, jbr * P:(jbr + 1) * P],
                                        ident[:])
                e1 = msb.tile([P, R], F32, tag="et")
                nc.scalar.activation(e1[:], s_i_rep[:], AF.Exp,
                                     bias=negs_col[:, jb:jb + 1])
                e01 = msb.tile([P, R], F32, tag="ee")
                nc.scalar.activation(e01[:], s_i_rep[:], AF.Exp, scale=0.1,
                                     bias=negs01_col[:, jb:jb + 1])
                ee = msb.tile([P, R], F32, tag="eemax")
                nc.vector.tensor_tensor(ee[:], e1[:], e01[:], ALU.max)
                srawT = msb.tile([P, R], F32, tag="srawT")
                nc.vector.tensor_tensor(srawT[:], pAT[:, 0:R], ee[:], ALU.mult)
                sq2 = msb.tile([P, R], F32, tag="sq2")
                nc.gpsimd.tensor_tensor(sq2[:], srawT[:], srawT[:], ALU.mult)
                first, last = jb == 0, jb == NJ - 1
                for f in range(0, R, NF):
                    nc.tensor.matmul(psum_y[:, f:f + NF],
                                     xaug[:, jb * C_AUG:(jb + 1) * C_AUG],
                                     srawT[:, f:f + NF], start=first,
                                     stop=last)
                    nc.tensor.matmul(psum_ssq[:, f:f + NF], ones128[:],
                                     sq2[:, f:f + NF], start=first, stop=last)

            ynrawT = persist.tile([H2, R], F32, tag="ynrawT")
            nc.scalar.copy(ynrawT[:], psum_y[0:H2, :])
            rho_raw = persist.tile([1, R], F32, tag="rho_raw")
            nc.scalar.copy(rho_raw[:], psum_y[H2:H2 + 1, :])
            snsq_raw = persist.tile([1, R], F32, tag="snsq_raw")
            nc.scalar.copy(snsq_raw[:], psum_y[C_AUG - 1:C_AUG, :])
            ssq_row = persist.tile([1, R], F32, tag="ssq_row")
            nc.scalar.copy(ssq_row[:], psum_ssq)

        # ---- phase 3: epilogue ----
        with ExitStack() as ctx3:
            r_rho = esb.tile([1, R], F32, tag="r_rho")
            nc.vector.reciprocal(r_rho[:], rho_raw[:])
            part_sb = esb.tile([1, 2], F32, tag="part_sb")
            nc.vector.tensor_tensor(sm_row[:], ssq_row[:], r_rho[:], ALU.mult)
            nc.vector.scalar_tensor_tensor(ssq_row[:], sm_row[:], 1.0,
                                           r_rho[:], ALU.mult, ALU.mult,
                                           accum_out=part_sb[:, 1:2])
            # head on RAW y with rank-1 rho*bias folded in (leaky is
            # positively homogeneous): z_s = leaky(Wg@y_raw + rho (x) bg)
            pz = big_a[0:H2, 0:R]
            for f in range(0, R, NF):
                nc.tensor.matmul(pz[:, f:f + NF], WgT, ynrawT[:, f:f + NF],
                                 start=True, stop=False)
                nc.tensor.matmul(pz[:, f:f + NF], bgr[:],
                                 rho_raw[:, f:f + NF], start=False, stop=True)
            dotm = esb.tile([H2, R], F32, tag="dotm")
            nc.scalar.copy(dotm[:], pz)
            zT = esb.tile([H2, R], F32, tag="zT")
            nc.scalar.activation(zT[:], pz, AF.Relu, scale=-1.0)
            nc.vector.scalar_tensor_tensor(zT[:], zT[:], 0.9, dotm[:],
                                           ALU.mult, ALU.add)
            po = big_b[0:D_OUT, 0:R]
            for f in range(0, R, NF):
                nc.tensor.matmul(po[:, f:f + NF], W2T, zT[:, f:f + NF],
                                 start=True, stop=False)
                nc.tensor.matmul(po[:, f:f + NF], b2r[:],
                                 rho_raw[:, f:f + NF], start=False, stop=True)
            oT = esb.tile([D_OUT, R], F32, tag="oT")
            nc.scalar.copy(oT[:], po)
            # rho reciprocal as per-i-tile columns via PE transposes
            _mini(nc, big_d[0:1, 512:513], ident)
            pcolr = big_d[:, 512:512 + NT]
            for t in range(NT):
                nc.tensor.transpose(pcolr[:, t:t + 1],
                                    r_rho[0:1, t * P:(t + 1) * P],
                                    ident[0:1, 0:1])
            rcol = esb.tile([P, NT], F32, tag="rcol")
            nc.scalar.copy(rcol[:], pcolr)
            _mini(nc, big_c[0:1, 0:1], ident)
            pon = big_c[:, 0:NT * D_OUT]
            for t in range(NT):
                nc.tensor.transpose(pon[:, t * D_OUT:(t + 1) * D_OUT],
                                    oT[:, t * P:(t + 1) * P],
                                    ident[0:D_OUT, 0:D_OUT])
            onat = esb.tile([P, NT * D_OUT], F32, tag="onat")
            for t in range(NT):
                nc.scalar.activation(onat[:, t * D_OUT:(t + 1) * D_OUT],
                                     pon[:, t * D_OUT:(t + 1) * D_OUT],
                                     AF.Copy, scale=rcol[:, t:t + 1])
            with tc.tile_critical(), nc.semaphore() as dsem_o:
                nc.sync.dma_start(
                    out_mine[:].rearrange("(t p) o -> p t o", p=P),
                    onat[:].rearrange("p (t o) -> p t o", o=D_OUT)
                ).then_inc(dsem_o, 16)
                nc.sync.wait_ge(dsem_o, 16)
            # L1 = sum(sq_mine + r_rho*(snsq_raw - (2/64)*dot_raw))
            nc.vector.tensor_tensor(dotm[:], xmT[:], ynrawT[:], ALU.mult)
            pdot = big_d[0:1, 0:R]
            for f in range(0, R, NF):
                nc.tensor.matmul(pdot[:, f:f + NF], ones64c[:],
                                 dotm[:, f:f + NF], start=True, stop=True)
            nc.vector.scalar_tensor_tensor(snsq_raw[:], pdot, -2.0 / H2,
                                           snsq_raw[:], ALU.mult, ALU.add)
            nc.vector.tensor_tensor(sm_row[:], snsq_raw[:], r_rho[:], ALU.mult)
            nc.vector.scalar_tensor_tensor(snsq_raw[:], sm_row[:], 1.0,
                                           sqm_row[:], ALU.mult, ALU.add,
                                           accum_out=part_sb[:, 0:1])
            with tc.tile_critical(), nc.semaphore() as dsem_p:
                nc.sync.dma_start(partials[:], part_sb[:]).then_inc(dsem_p, 16)
                nc.sync.wait_ge(dsem_p, 16)

    nc.finalize()
    return nc


_PROG_CACHE = {}


def _get_program(N, R):
    key = (N, R)
    if key not in _PROG_CACHE:
        _PROG_CACHE[key] = build_program(N, R)
    return _PROG_CACHE[key]


def make_in_maps(x, A, W1_0, b1_0, W1_1, b1_1, a, Wg_0, bg_0, W2_0, b2_0,
                 n_cores):
    x = np.ascontiguousarray(x, dtype=np.float32)
    A = np.ascontiguousarray(A, dtype=np.float32)
    N = x.shape[0]
    R = N // n_cores
    common = {
        "x_full": x,
        "W1_0": np.ascontiguousarray(W1_0, np.float32),
        "b1_0": np.ascontiguousarray(b1_0, np.float32).reshape(H1, 1),
        "W1_1": np.ascontiguousarray(W1_1, np.float32),
        "b1_1": np.ascontiguousarray(b1_1, np.float32).reshape(H2, 1),
        "a_vec": np.ascontiguousarray(a, np.float32).reshape(H2, 1),
        "Wg": np.ascontiguousarray(Wg_0, np.float32),
        "bg": np.ascontiguousarray(bg_0, np.float32).reshape(1, H2),
        "W2": np.ascontiguousarray(W2_0, np.float32),
        "b2": np.ascontiguousarray(b2_0, np.float32).reshape(1, D_OUT),
        "ident": np.eye(P, dtype=np.float32),
    }
    in_maps = []
    for c in range(n_cores):
        m = dict(common)
        m["A_sh"] = np.ascontiguousarray(A[c * R:(c + 1) * R])
        m["x_mine"] = np.ascontiguousarray(x[c * R:(c + 1) * R])
        in_maps.append(m)
    return in_maps


def kernel(x, A, W1_0, b1_0, W1_1, b1_1, a, Wg_0, bg_0, W2_0, b2_0):
    from concourse.bass_utils import run_bass_kernel_spmd

    n_cores = 8
    N = x.shape[0]
    R = N // n_cores
    nc = _get_program(N, R)
    in_maps = make_in_maps(x, A, W1_0, b1_0, W1_1, b1_1, a, Wg_0, bg_0,
                           W2_0, b2_0, n_cores)
    res = run_bass_kernel_spmd(nc, in_maps, list(range(n_cores)))
    outs = res.results
    out = np.concatenate([outs[c]["out_mine"] for c in range(n_cores)], axis=0)
    l1 = sum(float(outs[c]["partials"][0, 0]) for c in range(n_cores)) / N**2
    l2 = sum(float(outs[c]["partials"][0, 1]) for c in range(n_cores)) / N**2
    return out, np.float32(l1), np.float32(l2)
